# revision 13
# baseline (speedup 1.0000x reference)
"""Trainium2 Bass kernel for nn_FGCN: two single-layer GCNConvs (normalize=False).

  emb1 = segment_sum((drug_x @ W1)[src1], dst1, N_DRUG) + b1
  emb2 = segment_sum((dis_x  @ W2)[src2], dst2, N_DIS)  + b2

Strategy (8 NeuronCores, SPMD):
  - Destination nodes sharded across cores; host buckets edges by
    (core, 128-dst window), pads each window to a uniform T chunks of
    128 edges (same T on every core -> one SPMD program).
  - Phase 1 (sharded): each core computes h = x @ W for its 1/8 node
    shard (host supplies per-core x^T shards so lhsT tiles DMA
    directly), then AllGather assembles the full h table on every core.
  - Phase 2: per window, dma_gather fetches all T*128 source rows of
    h; one broadcast is_equal against an iota tile builds the
    [128e, 128d] one-hot selection matrices; T matmuls accumulate
    onehot^T @ msg into PSUM == exact scatter-add; add bias; DMA out.
  - dma_gather indexes are int16: graphs with too many nodes split the
    h table by src parity (even/odd) so indices stay < 32768; each
    window's edges are grouped into even-chunks then odd-chunks and
    gathered from the matching table.
"""

import numpy as np

P = 128
H = 64
NCORES = 8
N1, F1 = 50000, 256
N2, F2 = 20000, 128

LAST_RESULTS = None  # BassKernelResults of the most recent device run
PAIR_THRESHOLD = 32767  # above this node count, split h by src parity


def _prep_graph(edge_index, n_nodes, n_cores):
    """Host-side edge partitioning. Returns a cfg dict with per-core arrays."""
    edge_index = np.asarray(edge_index)
    src = edge_index[0].astype(np.int64)
    dst = edge_index[1].astype(np.int64)
    assert n_nodes % n_cores == 0
    per = n_nodes // n_cores          # output (dst) shard per core
    nwin = -(-per // P)
    npad = -(-n_nodes // 1024) * 1024  # h-table node padding
    S = npad // n_cores               # phase-1 node shard per core
    pair = n_nodes > PAIR_THRESHOLD
    if pair:
        half = S // 2
        half_pad = -(-half // P) * P  # padded parity rows per core shard
        tab_rows = n_cores * half_pad  # rows of each parity table
        shard_rows = 2 * half_pad
    else:
        half = half_pad = 0
        tab_rows = npad
        shard_rows = S
    assert tab_rows <= 32768

    core = dst // per
    dstl = dst - core * per
    win = dstl >> 7
    loc = (dstl & 127).astype(np.float32)
    gid = core * nwin + win
    if pair:
        par = (src & 1).astype(np.int64)
        c_s = src // S
        idxv = (c_s * half_pad + ((src - c_s * S) >> 1)).astype(np.int16)
    else:
        par = np.zeros_like(src)
        idxv = src.astype(np.int16)

    kid = gid * 2 + par
    order = np.lexsort((src, kid))
    kid_s = kid[order]
    cnt2 = np.bincount(kid_s, minlength=n_cores * nwin * 2)
    Te = int(np.ceil(cnt2[0::2].max() / P))
    To = int(np.ceil(cnt2[1::2].max() / P)) if pair else 0
    T = Te + To
    starts2 = np.zeros(len(cnt2) + 1, np.int64)
    np.cumsum(cnt2, out=starts2[1:])
    rank = np.arange(len(src)) - starts2[kid_s]
    par_s = kid_s & 1
    gid_s = kid_s >> 1
    w_s = gid_s % nwin
    c_s2 = gid_s // nwin
    chunk = np.where(par_s == 1, Te + (rank >> 7), rank >> 7)
    prt = rank & 127

    # dstloc [cores, 128, nwin*T] f32, -1 pads
    dl = np.full((n_cores, P, nwin * T), -1.0, np.float32)
    dl[c_s2, prt, w_s * T + chunk] = loc[order]

    # gather idx table: unwrapped rank-within-window j = chunk*128 + prt,
    # wrapped to partition j%16 / column j//16, replicated across the 8
    # 16-partition groups. Pads gather row 0 (always valid; their one-hot
    # column is zero so they drop out of the matmul).
    U = np.zeros((n_cores, nwin * T * P), np.int16)
    U[c_s2, (w_s * T + chunk) * P + prt] = idxv[order]
    ig = np.tile(
        U.reshape(n_cores, nwin * T * 8, 16).transpose(0, 2, 1), (1, 8, 1)
    )  # [cores, 128, nwin*T*8]

    return dict(
        n_nodes=n_nodes,
        per=per,
        nwin=nwin,
        npad=npad,
        S=S,
        pair=pair,
        half=half,
        half_pad=half_pad,
        tab_rows=tab_rows,
        shard_rows=shard_rows,
        Te=Te,
        To=To,
        T=T,
        dl=dl,
        ig=np.ascontiguousarray(ig),
    )


def _prep_xT_shards(x, cfg, n_cores):
    """Per-core x^T shards in phase-1 node order (parity-permuted if pair)."""
    x = np.asarray(x, np.float32)
    n, F = x.shape
    S, R = cfg["S"], cfg["shard_rows"]
    out = np.zeros((n_cores, F, R), np.float32)
    for c in range(n_cores):
        lo, hi = c * S, min((c + 1) * S, n)
        blk = x[lo:hi]
        if cfg["pair"]:
            hp = cfg["half_pad"]
            ev = blk[0::2]
            od = blk[1::2]
            out[c, :, : ev.shape[0]] = ev.T
            out[c, :, hp : hp + od.shape[0]] = od.T
        else:
            out[c, :, : hi - lo] = blk.T
    return out


def _build_program(cfgs, feats, n_cores):
    import concourse.bass as bass
    import concourse.mybir as mybir
    import concourse.tile as tile
    from concourse import bacc

    f32 = mybir.dt.float32
    i16 = mybir.dt.int16

    nc = bacc.Bacc("TRN2", target_bir_lowering=False)
    dram = []
    for g, (cfg, F) in enumerate(zip(cfgs, feats)):
        nwin, T, R = cfg["nwin"], cfg["T"], cfg["shard_rows"]
        tabs = {}
        if cfg["pair"]:
            tabs["hev"] = nc.dram_tensor(
                f"hev{g}", [cfg["tab_rows"], H], f32, addr_space="Shared"
            )
            tabs["hod"] = nc.dram_tensor(
                f"hod{g}", [cfg["tab_rows"], H], f32, addr_space="Shared"
            )
        else:
            tabs["h"] = nc.dram_tensor(
                f"h{g}", [cfg["tab_rows"], H], f32, addr_space="Shared"
            )
        dram.append(
            dict(
                xT=nc.dram_tensor(f"xT{g}", [F, R], f32, kind="ExternalInput"),
                W=nc.dram_tensor(f"W{g}", [F, H], f32, kind="ExternalInput"),
                B=nc.dram_tensor(f"B{g}", [P, H], f32, kind="ExternalInput"),
                IG=nc.dram_tensor(
                    f"IG{g}", [P, nwin * T * 8], i16, kind="ExternalInput"
                ),
                DL=nc.dram_tensor(f"DL{g}", [P, nwin * T], f32, kind="ExternalInput"),
                O=nc.dram_tensor(f"O{g}", [nwin * P, H], f32, kind="ExternalOutput"),
                **tabs,
            )
        )
    IOTAd = nc.dram_tensor("IOTAd", [P, P], f32, kind="ExternalInput")
    rg = [list(range(n_cores))]

    with tile.TileContext(nc) as tc:
        with (
            tc.tile_pool(name="const", bufs=1) as cpool,
            tc.tile_pool(name="hshard", bufs=1, space="DRAM") as dpool,
            tc.tile_pool(name="xin", bufs=3) as xpool,
            tc.tile_pool(name="hout", bufs=3) as hpool,
            tc.tile_pool(name="psumh", bufs=2, space="PSUM") as psumh,
            tc.tile_pool(name="msg", bufs=3) as msgpool,
            tc.tile_pool(name="oh", bufs=3) as ohpool,
            tc.tile_pool(name="acc", bufs=4, space="PSUM") as accpool,
            tc.tile_pool(name="ob", bufs=3) as obpool,
        ):
            iota = cpool.tile([P, P], f32, tag="iota")
            nc.sync.dma_start(out=iota[:], in_=IOTAd[:, :])
            consts = []
            for g, (cfg, F, dr) in enumerate(zip(cfgs, feats, dram)):
                KC = F // P
                wts = []
                for k in range(KC):
                    wt = cpool.tile([P, H], f32, tag=f"w{g}_{k}")
                    nc.sync.dma_start(out=wt[:], in_=dr["W"][k * P : (k + 1) * P, :])
                    wts.append(wt)
                bt = cpool.tile([P, H], f32, tag=f"b{g}")
                nc.sync.dma_start(out=bt[:], in_=dr["B"][:, :])
                igt = cpool.tile([P, cfg["nwin"] * cfg["T"] * 8], i16, tag=f"ig{g}")
                nc.sync.dma_start(out=igt[:], in_=dr["IG"][:, :])
                dlt = cpool.tile([P, cfg["nwin"] * cfg["T"]], f32, tag=f"dl{g}")
                nc.sync.dma_start(out=dlt[:], in_=dr["DL"][:, :])
                consts.append((wts, bt, igt, dlt))

            # ---- phase 1: h shard = x_shard @ W, blocks of <=5 psum chunks --
            # (iterate graphs in reverse so graph 0's AllGather — the first
            # thing phase 2 needs — is the last collective issued)
            for g, (cfg, F, dr) in reversed(list(enumerate(zip(cfgs, feats, dram)))):
                KC = F // P
                wts = consts[g][0]
                R = cfg["shard_rows"]
                hs_all = dpool.tile([R, H], f32, tag=f"hs{g}")
                nchunks = R // P
                b0 = 0
                while b0 < nchunks:
                    Q = min(5, nchunks - b0)
                    n0 = b0 * P
                    xts = []
                    for k in range(KC):
                        xt = xpool.tile([P, 5 * P], f32, tag=f"x{g}_{k}")
                        nc.sync.dma_start(
                            out=xt[:, : Q * P],
                            in_=dr["xT"][k * P : (k + 1) * P, n0 : n0 + Q * P],
                        )
                        xts.append(xt)
                    ph = psumh.tile([P, 5 * H], f32, tag="ph")
                    for q in range(Q):
                        for k in range(KC):
                            nc.tensor.matmul(
                                out=ph[:, q * H : (q + 1) * H],
                                lhsT=xts[k][:, q * P : (q + 1) * P],
                                rhs=wts[k][:],
                                start=(k == 0),
                                stop=(k == KC - 1),
                            )
                    hs = hpool.tile([P, 5 * H], f32, tag="hs")
                    nc.vector.tensor_copy(out=hs[:, : Q * H], in_=ph[:, : Q * H])
                    dst_ap = (
                        hs_all[n0 : n0 + Q * P, :]
                        .rearrange("(q p) h -> q p h", p=P)
                        .transpose([1, 0, 2])
                    )
                    nc.sync.dma_start(
                        out=dst_ap,
                        in_=hs[:, : Q * H].rearrange("p (q h) -> p q h", q=Q),
                    )
                    b0 += Q

                hp = cfg["half_pad"]
                if cfg["pair"]:
                    nc.gpsimd.collective_compute(
                        "AllGather",
                        mybir.AluOpType.bypass,
                        ins=[hs_all[0:hp, :].opt()],
                        outs=[dr["hev"][:]],
                        replica_groups=rg,
                    )
                    nc.gpsimd.collective_compute(
                        "AllGather",
                        mybir.AluOpType.bypass,
                        ins=[hs_all[hp : 2 * hp, :].opt()],
                        outs=[dr["hod"][:]],
                        replica_groups=rg,
                    )
                else:
                    nc.gpsimd.collective_compute(
                        "AllGather",
                        mybir.AluOpType.bypass,
                        ins=[hs_all[:].opt()],
                        outs=[dr["h"][:]],
                        replica_groups=rg,
                    )

            # phase 2 gathers may read any h row: fence on full h tables
            tc.strict_bb_all_engine_barrier()

            # ---- phase 2: gather + one-hot matmul scatter-add per window ----
            # interleave the graphs' windows so the per-window engine mix
            # (drug: DMA-heavier; dis: DVE-heavier) evens out over time
            sched = []
            for g, cfg in enumerate(cfgs):
                nw = cfg["nwin"]
                for i, w in enumerate(range(nw)):
                    sched.append((i * (1.0 / nw), g, w))
            sched.sort()
            for _, g, w in sched:
                cfg, F, dr = cfgs[g], feats[g], dram[g]
                wts, bt, igt, dlt = consts[g]
                nwin, T, Te, pair = cfg["nwin"], cfg["T"], cfg["Te"], cfg["pair"]
                if True:
                    msg = msgpool.tile([P, T * H], f32, tag=f"msg{g}")
                    if pair:
                        parts = [
                            (0, Te, dr["hev"]),
                            (Te, T, dr["hod"]),
                        ]
                    else:
                        parts = [(0, T, dr["h"])]
                    for t0, t1, tab in parts:
                        nt = t1 - t0
                        if nt == 0:
                            continue
                        nc.gpsimd.dma_gather(
                            out_ap=msg[:, t0 * H : t1 * H].rearrange(
                                "p (t e) -> p t e", t=nt
                            ),
                            in_ap=tab[:],
                            idxs_ap=igt[
                                :, (w * T + t0) * 8 : (w * T + t1) * 8
                            ],
                            num_idxs=nt * P,
                            num_idxs_reg=nt * P,
                            elem_size=H,
                            single_packet=(nt * P <= 1024),
                        )
                    oh = ohpool.tile([P, T * P], f32, tag=f"oh{g}")
                    nc.vector.tensor_tensor(
                        out=oh[:].rearrange("p (t j) -> p t j", t=T),
                        in0=dlt[:, w * T : (w + 1) * T]
                        .unsqueeze(2)
                        .to_broadcast([P, T, P]),
                        in1=iota[:].unsqueeze(1).to_broadcast([P, T, P]),
                        op=mybir.AluOpType.is_equal,
                    )
                    acc = accpool.tile([P, H], f32, tag="acc")
                    for t in range(T):
                        nc.tensor.matmul(
                            out=acc[:],
                            lhsT=oh[:, t * P : (t + 1) * P],
                            rhs=msg[:, t * H : (t + 1) * H],
                            start=(t == 0),
                            stop=(t == T - 1),
                        )
                    ob = obpool.tile([P, H], f32, tag="ob")
                    nc.vector.tensor_add(out=ob[:], in0=acc[:], in1=bt[:])
                    nc.sync.dma_start(
                        out=dr["O"][w * P : (w + 1) * P, :], in_=ob[:]
                    )

    nc.finalize()
    return nc


def run_graphs(graphs, n_cores=NCORES, core_ids=None, trace=False):
    """graphs: list of dicts {x, W, b, edge_index, n_nodes}. Returns list of
    full [n_nodes, H] outputs."""
    global LAST_RESULTS
    from concourse.bass_utils import run_bass_kernel_spmd

    cfgs = [_prep_graph(g["edge_index"], g["n_nodes"], n_cores) for g in graphs]
    feats = [np.asarray(g["x"]).shape[1] for g in graphs]
    nc = _build_program(cfgs, feats, n_cores)

    common = {
        "IOTAd": np.ascontiguousarray(np.tile(np.arange(P, dtype=np.float32), (P, 1)))
    }
    xshards = []
    for g, (cfg, gr) in enumerate(zip(cfgs, graphs)):
        xshards.append(_prep_xT_shards(gr["x"], cfg, n_cores))
        common[f"W{g}"] = np.ascontiguousarray(np.asarray(gr["W"], np.float32))
        common[f"B{g}"] = np.ascontiguousarray(
            np.tile(np.asarray(gr["b"], np.float32), (P, 1))
        )
    in_maps = []
    for c in range(n_cores):
        m = dict(common)
        for g, cfg in enumerate(cfgs):
            m[f"xT{g}"] = np.ascontiguousarray(xshards[g][c])
            m[f"IG{g}"] = np.ascontiguousarray(cfg["ig"][c])
            m[f"DL{g}"] = np.ascontiguousarray(cfg["dl"][c])
        in_maps.append(m)

    if core_ids is None:
        core_ids = list(range(n_cores))
    res = run_bass_kernel_spmd(nc, in_maps, core_ids, trace=trace)
    LAST_RESULTS = res

    outs = []
    for g, cfg in enumerate(cfgs):
        per = cfg["per"]
        outs.append(
            np.concatenate([res.results[c][f"O{g}"][:per] for c in range(n_cores)])
        )
    return outs


def kernel(drug_x, dis_x, W1, b1, W2, b2, drug_edge_index, dis_edge_index):
    emb1, emb2 = run_graphs(
        [
            dict(x=drug_x, W=W1, b=b1, edge_index=drug_edge_index, n_nodes=N1),
            dict(x=dis_x, W=W2, b=b2, edge_index=dis_edge_index, n_nodes=N2),
        ]
    )
    return emb1, emb2
